# revision 1
# baseline (speedup 1.0000x reference)
"""GroupedQueryAttention kernel for 8 Trainium2 NeuronCores (~245us HW).

Sharding: tensor-parallel over KV groups (core c owns group c = 4 query
heads x 64): column shards of w_q/w_k/w_v, row shard of w_o; x
replicated (bf16, pre-transposed, partition-major); each core writes a
partial f32 output that the host sums.

Design (vs the 2261us f32r baseline):
- all matmul operands bf16 (rel err ~5.5e-3, well under the 2e-2 gate)
- head-paired score matmuls on disjoint PE row groups run concurrently
  (K^T duplicated on both partition halves; Q/O heads live at
  partitions 64*(h%2), slot h//2)
- one exp ACTIVATE per sk tile covers both heads ([128,2,512] PSUM)
- softmax denominator via the [V | ones] column trick; AV psum banks
  are evacuated immediately (denominator row + raw O to SBUF) so the
  accumulators free fast, and normalization (reciprocal_approx_fast +
  ones-row broadcast matmul + DVE mul) runs off the critical path
- one flat software pipeline over all (chunk, pair, sk) items: scores
  are emitted 2 items ahead, the previous pair's epilogue fires behind
  the next pair's scores, o-proj of chunk ch-1 and (in chunk 0) the
  Q(qt1) projection + V transposes interleave as PE filler work
- host-side partition-major layouts make every DMA a contiguous
  [128, N] copy; projections stream against the xT tile DMAs

Layouts per core (S=2048, D=2048, 4 heads of 64):
  xT_sb  [128, 16, 2048] bf16   x^T k-tiles (host partition-major)
  qT_sb  [128, 2, 2048]  bf16   Q^T; head h -> partitions 64*(h%2), slot h//2
  kT_sb  [128, 2048]     bf16   K^T duplicated on both partition halves
  v1_sb  [128, 16, 65]   bf16   [V | ones] natural layout per sk tile
  oT_sb  [128, 2, 2048]  bf16   normalized attention out (same map as qT)
  out    [2048, 2048]    f32    partial output, host-summed
"""

import numpy as np

S = 2048
D = 2048
N_CORES = 8
HD = 64
HPG = 4
QDIM = HPG * HD           # 256
SCALE = 1.0 / 8.0         # 1/sqrt(HD)
SQC = 512                 # seq chunk (psum bank width in f32)
NCH = S // SQC            # 4
T = S // 128              # 16 sk tiles
KO = D // 128             # 16 contraction tiles
QT = QDIM // 128          # 2 q partition tiles (= head pairs)

_compiled = {}


def _noldw(bi):
    """Mark a matmul as non-self-loading (reuses the PE array weights
    loaded by the immediately preceding matmul on the Tensor engine)."""
    bi.ins.ldweights = False
    return bi


def build_gqa(debug=False):
    import concourse.tile as tile
    from concourse import bacc, mybir
    from concourse.masks import make_identity
    from contextlib import ExitStack

    f32 = mybir.dt.float32
    bf16 = mybir.dt.bfloat16
    EXP = mybir.ActivationFunctionType.Exp

    nc = bacc.Bacc(None, target_bir_lowering=False, debug=debug)
    # host provides partition-major layouts: [p, ...] with p the SBUF partition
    xTp = nc.declare_dram_parameter("xTp", [128, KO * S], bf16, isOutput=False)
    wqp = nc.declare_dram_parameter("wqp", [128, KO * QDIM], bf16, isOutput=False)
    wkvp = nc.declare_dram_parameter("wkvp", [128, KO * 2 * HD], bf16, isOutput=False)
    wop = nc.declare_dram_parameter("wop", [128, QT * D], bf16, isOutput=False)
    out = nc.declare_dram_parameter("out", [S, D], bf16, isOutput=True)

    with tile.TileContext(nc) as tc, ExitStack() as ctx:
        const = ctx.enter_context(tc.tile_pool(name="const", bufs=1))
        persist = ctx.enter_context(tc.tile_pool(name="persist", bufs=1))

        ident = const.tile([128, 128], bf16)
        make_identity(nc, ident)
        ones_bf = const.tile([1, HD], bf16)
        nc.vector.memset(ones_bf, 1.0)
        bias_exp = const.tile([128, 1], f32)
        nc.vector.memset(bias_exp, -8.0)

        xT_sb = persist.tile([128, KO, S], bf16)
        qT_sb = persist.tile([128, QT, S], bf16)
        kT_sb = persist.tile([128, S], bf16)
        v1_sb = persist.tile([128, T, HD + 1], bf16)
        oT_sb = persist.tile([128, QT, S], bf16)
        wkv_sb = persist.tile([128, KO, 2 * HD], bf16)
        wq_sb = persist.tile([128, KO, QDIM], bf16)
        wo_sb = persist.tile([128, QT, D], bf16)

        nc.vector.memset(v1_sb[:, :, HD:HD + 1], 1.0)

        # ---------------- input DMAs (contiguous per partition) ----------
        # order matters: the first kv matmul needs only wkv + xT[0]; wq is
        # needed ~1us later, wo not until chunk 1's o-proj
        nc.sync.dma_start(out=wkv_sb, in_=wkvp[:].rearrange("p (ko m) -> p ko m", ko=KO))
        for ko in range(2):
            nc.sync.dma_start(
                out=xT_sb[:, ko, :], in_=xTp[:, ko * S:(ko + 1) * S])
        nc.sync.dma_start(out=wq_sb, in_=wqp[:].rearrange("p (ko m) -> p ko m", ko=KO))
        for ko in range(2, KO):
            nc.sync.dma_start(
                out=xT_sb[:, ko, :], in_=xTp[:, ko * S:(ko + 1) * S])
        nc.sync.dma_start(out=wo_sb, in_=wop[:].rearrange("p (qt m) -> p qt m", qt=QT))

        # ---------------- prologue: KV + Q(qt0) projections ----------------
        # One DMA-paced sweep; Q(qt1) and the V transposes run later as
        # chunk-0 fillers inside phase 2.
        pev = ctx.enter_context(tc.tile_pool(name="pev", bufs=2))
        vT_tmp = pev.tile([64, S], bf16, name="vT_tmp", tag="vt")
        with tc.tile_pool(name="ppool", bufs=8, space="PSUM") as pp:
            kv_ps = [pp.tile([128, SQC], f32, name=f"kv{ch}", tag="pp")
                     for ch in range(NCH)]
            q0_ps = [pp.tile([128, SQC], f32, name=f"q0{ch}", tag="pp")
                     for ch in range(NCH)]
            for ko in range(KO):
                for ch in range(NCH):
                    cs = slice(ch * SQC, (ch + 1) * SQC)
                    nc.tensor.matmul(
                        kv_ps[ch], wkv_sb[:, ko, :], xT_sb[:, ko, cs],
                        start=(ko == 0), stop=(ko == KO - 1))
                for ch in range(NCH):
                    cs = slice(ch * SQC, (ch + 1) * SQC)
                    nc.tensor.matmul(
                        q0_ps[ch], wq_sb[:, ko, 0:128], xT_sb[:, ko, cs],
                        start=(ko == 0), stop=(ko == KO - 1))

            for ch in range(NCH):
                cs = slice(ch * SQC, (ch + 1) * SQC)
                nc.vector.tensor_copy(out=kT_sb[0:64, cs], in_=kv_ps[ch][0:64, :])
                nc.vector.tensor_copy(out=kT_sb[64:128, cs], in_=kv_ps[ch][0:64, :])
                nc.vector.tensor_copy(out=vT_tmp[:, cs], in_=kv_ps[ch][64:128, :])
            for ch in range(NCH):
                cs = slice(ch * SQC, (ch + 1) * SQC)
                nc.vector.tensor_copy(out=qT_sb[:, 0, cs], in_=q0_ps[ch])

        # ---------------- phase 2: attention + o-proj, one flat pipeline --
        scps = ctx.enter_context(tc.tile_pool(name="scps", bufs=2, space="PSUM"))
        avps = ctx.enter_context(tc.tile_pool(name="avps", bufs=2, space="PSUM"))
        mips = ctx.enter_context(tc.tile_pool(name="mips", bufs=2, space="PSUM"))
        eps = ctx.enter_context(tc.tile_pool(name="eps", bufs=8))
        p2ev = ctx.enter_context(tc.tile_pool(name="p2ev", bufs=4))
        ypool = ctx.enter_context(tc.tile_pool(name="ypool", bufs=4))

        def emit_oproj_task(t, och, tail_idx=-1):
            """o-proj for seq tile t, one output column chunk."""
            ns = slice(och * SQC, (och + 1) * SQC)
            if tail_idx >= 0:
                # tail: attention pools are idle — rotate across all of them
                pool, tag = [(mips, "mip"), (avps, "av"), (scps, "sc")][tail_idx % 3]
                py = pool.tile([128, SQC], f32, name="py", tag=tag)
            else:
                py = mips.tile([128, SQC], f32, name="py", tag="mip")
            for qt in range(QT):
                nc.tensor.matmul(
                    py, oT_sb[:, qt, t * 128:(t + 1) * 128], wo_sb[:, qt, ns],
                    start=(qt == 0), stop=(qt == QT - 1))
            y_sb = ypool.tile([128, SQC], bf16, name="y_sb")
            with nc.allow_low_precision(reason="bf16 partial output"):
                if tail_idx >= 0 and tail_idx % 2 == 1:
                    nc.scalar.copy(out=y_sb, in_=py)
                else:
                    nc.vector.tensor_copy(out=y_sb, in_=py)
            nc.sync.dma_start(
                out=out[:].rearrange("(t p) n -> p t n", p=128)[:, t, ns],
                in_=y_sb)

        def evacuate_av(av):
            """Free the AV psum banks fast: pull denominator + raw O to SBUF.
            Normalization happens later, off the AV-accumulator critical path."""
            den, orw = [], []
            for hh in range(2):
                den.append(p2ev.tile([1, SQC], f32, name=f"den{hh}", tag=f"den{hh}"))
                nc.vector.tensor_copy(out=den[hh], in_=av[hh][HD:HD + 1, :])
                orw.append(p2ev.tile([HD, SQC], bf16, name=f"orw{hh}", tag=f"orw{hh}"))
                with nc.allow_low_precision(reason="bf16 attn out"):
                    nc.vector.tensor_copy(out=orw[hh], in_=av[hh][0:HD, :])
            return den, orw

        def make_epilogue(ch, qt, den, orw):
            cs = slice(ch * SQC, (ch + 1) * SQC)

            def epi():
                for hh in range(2):
                    rf = p2ev.tile([1, SQC], f32, name=f"rf{hh}", tag=f"rf{hh}")
                    with nc.allow_low_precision(reason="softmax recip ~51ulp"):
                        nc.vector.reciprocal_approx_fast(out=rf, in_=den[hh])
                    rec = p2ev.tile([1, SQC], bf16, name=f"rec{hh}", tag=f"rec{hh}")
                    with nc.allow_low_precision(reason="bf16 recip bcast"):
                        nc.vector.tensor_copy(out=rec, in_=rf)
                    bc = mips.tile([128, SQC], f32, name="bc", tag="mip")
                    nc.tensor.matmul(
                        bc[0:HD, :], ones_bf, rec, start=True, stop=True)
                    bc_sb = p2ev.tile([HD, SQC], bf16, name=f"bcs{hh}", tag=f"bcs{hh}")
                    with nc.allow_low_precision(reason="bf16 recip bcast"):
                        nc.vector.tensor_copy(out=bc_sb, in_=bc[0:HD, :])
                    with nc.allow_low_precision(reason="bf16 attn out"):
                        nc.vector.tensor_mul(
                            out=oT_sb[64 * hh:64 * hh + 64, qt, cs],
                            in0=orw[hh], in1=bc_sb)
            return epi

        def emit_vtrans(j):
            pt = mips.tile([128, HD], bf16, name="pt", tag="mip")
            nc.tensor.transpose(
                pt, vT_tmp[:, j * 128:(j + 1) * 128], ident[0:64, 0:64])
            nc.vector.tensor_copy(out=v1_sb[:, j, 0:HD], in_=pt)

        q1_state = {}

        def emit_q1_slice(c, j):
            """Quarter (4 ko) of the Q(qt1) projection for chunk c."""
            cs = slice(c * SQC, (c + 1) * SQC)
            if j == 0:
                q1_state[c] = mips.tile([128, SQC], f32, name="q1", tag="mip")
            q1 = q1_state[c]
            for ko in range(4 * j, 4 * j + 4):
                nc.tensor.matmul(
                    q1, wq_sb[:, ko, 128:256], xT_sb[:, ko, cs],
                    start=(ko == 0), stop=(ko == KO - 1))
            if j == 3:
                nc.vector.tensor_copy(out=qT_sb[:, 1, cs], in_=q1_state.pop(c))

        items = [(ch, qt, sk)
                 for ch in range(NCH) for qt in range(QT) for sk in range(T)]
        sc_tiles = {}
        vt_next = 0

        def emit_scores(idx):
            ch, qt, sk = items[idx]
            cs = slice(ch * SQC, (ch + 1) * SQC)
            sc = scps.tile([128, 2, SQC], f32, name="sc", tag="sc")
            for hh in range(2):
                hp = 64 * hh
                nc.tensor.matmul(
                    sc[:, hh, :],
                    kT_sb[hp:hp + 64, sk * 128:(sk + 1) * 128],
                    qT_sb[hp:hp + 64, qt, cs],
                    start=True, stop=True)
            sc_tiles[idx] = sc

        pending_epi = None
        fillers = []
        av = None
        # PE work that depends only on wq/xT: keeps the PE dense (and the
        # HAM clock warm) while the DVE evacuates the projection psums
        emit_q1_slice(0, 0)
        emit_q1_slice(0, 1)
        for idx, (ch, qt, sk) in enumerate(items):
            if sk == 0:
                if idx == 0:
                    emit_scores(0)
                    emit_scores(1)
                av = [avps.tile([HD + 1, SQC], f32, name=f"av{hh}", tag="av")
                      for hh in range(2)]
                if qt == 0:  # new chunk: queue o-proj of previous chunk
                    if ch > 0:
                        base = (ch - 1) * (SQC // 128)
                        fillers = [
                            (lambda t=base + tt, oc=oc: emit_oproj_task(t, oc))
                            for tt in range(SQC // 128) for oc in range(NCH)
                        ]
            e_sb = eps.tile([128, 2, SQC], bf16, name="e_sb")
            nc.scalar.activation(
                out=e_sb, in_=sc_tiles.pop(idx),
                func=EXP, bias=bias_exp, scale=1.0)
            if idx + 2 < len(items):
                emit_scores(idx + 2)
            if pending_epi is not None:
                pending_epi()
                pending_epi = None
            if ch == 0 and qt == 0:
                while vt_next <= min(sk + 2, T - 1):
                    emit_vtrans(vt_next)
                    vt_next += 1
                if sk < 14:  # slices 0,1 pre-emitted before the item loop
                    emit_q1_slice((sk + 2) // 4, (sk + 2) % 4)
            for hh in range(2):
                bi = nc.tensor.matmul(
                    av[hh][:, :], v1_sb[:, sk, :], e_sb[:, hh, :],
                    start=(sk == 0), stop=(sk == T - 1))
                if hh:
                    _noldw(bi)
            if sk % 2 == 1 and fillers:
                fillers.pop(0)()
            if sk == T - 1:
                den, orw = evacuate_av(av)
                pending_epi = make_epilogue(ch, qt, den, orw)
        pending_epi()
        for f in fillers:
            f()
        # tail: o-proj of last chunk in waves of 6 — all of a wave's
        # matmuls stream back-to-back before any evacuation, keeping the
        # PE dense enough that the clock stays warm
        base = (NCH - 1) * (SQC // 128)
        tasks = [(base + tt, oc) for tt in range(SQC // 128) for oc in range(NCH)]
        i = 0
        for w in range(0, len(tasks), 6):
            wave = tasks[w:w + 6]
            pys = []
            for t, oc in wave:
                ns = slice(oc * SQC, (oc + 1) * SQC)
                pool, tag = [(mips, "mip"), (avps, "av"), (scps, "sc")][i % 3]
                py = pool.tile([128, SQC], f32, name="py", tag=tag)
                for qt in range(QT):
                    nc.tensor.matmul(
                        py, oT_sb[:, qt, t * 128:(t + 1) * 128],
                        wo_sb[:, qt, ns],
                        start=(qt == 0), stop=(qt == QT - 1))
                pys.append(py)
                i += 1
            for (t, oc), py in zip(wave, pys):
                ns = slice(oc * SQC, (oc + 1) * SQC)
                y_sb = ypool.tile([128, SQC], bf16, name="y_sb")
                with nc.allow_low_precision(reason="bf16 partial output"):
                    if (t + oc) % 2 == 1:
                        nc.scalar.copy(out=y_sb, in_=py)
                    else:
                        nc.vector.tensor_copy(out=y_sb, in_=py)
                nc.sync.dma_start(
                    out=out[:].rearrange("(t p) n -> p t n", p=128)[:, t, ns],
                    in_=y_sb)

    nc.compile()
    return nc


def _get_nc():
    if "nc" not in _compiled:
        _compiled["nc"] = build_gqa()
    return _compiled["nc"]


def _pm(a):
    """[KO*128, M] -> partition-major [128, KO*M] (row p holds all ko chunks)."""
    ko = a.shape[0] // 128
    return np.ascontiguousarray(
        a.reshape(ko, 128, a.shape[1]).transpose(1, 0, 2).reshape(128, -1))


def _shard_inputs(x, w_q, w_k, w_v, w_o):
    import ml_dtypes

    bf = ml_dtypes.bfloat16
    x = np.asarray(x, dtype=np.float32)
    w_q = np.asarray(w_q, dtype=np.float32)
    w_k = np.asarray(w_k, dtype=np.float32)
    w_v = np.asarray(w_v, dtype=np.float32)
    w_o = np.asarray(w_o, dtype=np.float32)
    xT = np.ascontiguousarray(x.reshape(S, D).T)
    xTp = _pm(xT).astype(bf)
    in_maps = []
    for c in range(N_CORES):
        wkv = np.concatenate(
            [w_k[:, c * HD:(c + 1) * HD], w_v[:, c * HD:(c + 1) * HD]], axis=1)
        in_maps.append({
            "xTp": xTp,
            "wqp": _pm(w_q[:, c * QDIM:(c + 1) * QDIM] * np.float32(SCALE)).astype(bf),
            "wkvp": _pm(wkv).astype(bf),
            "wop": _pm(w_o[c * QDIM:(c + 1) * QDIM, :]).astype(bf),
        })
    return in_maps


def kernel(x, w_q, w_k, w_v, w_o):
    from concourse.bass_utils import run_bass_kernel_spmd

    nc = _get_nc()
    in_maps = _shard_inputs(x, w_q, w_k, w_v, w_o)
    res = run_bass_kernel_spmd(nc, in_maps, list(range(N_CORES)))
    acc = np.zeros((S, D), dtype=np.float64)
    for r in res.results:
        acc += r["out"].astype(np.float64)
    return acc.astype(np.float32).reshape(1, S, D)



# revision 5
# speedup vs baseline: 1.0117x; 1.0117x over previous
"""GroupedQueryAttention kernel for 8 Trainium2 NeuronCores.

Sharding: tensor-parallel over KV groups (core c owns group c = 4 query
heads x 64): column shards of w_q/w_k/w_v, row shard of w_o; x
replicated (bf16, pre-transposed, partition-major); each core writes a
partial bf16 output that the host sums.

v2 design (vs the 290us flat-pipeline baseline):
- The ACT (Scalar) engine's softmax exp stream is the long pole:
  128 ACTIVATEs x ~1.15us = ~147us that cannot be reduced (exp is
  ACT-only, 1 elem/cycle/lane).  So the kernel starts that stream as
  early as the DMA allows (~19us instead of ~55us) and hides ALL other
  PE work (KV/Q projections for chunks 1-3, Q(qt1), V transposes,
  o-proj) inside the exp-paced slack via a static DMA-aware filler
  schedule.
- x is DMA'd seq-chunk-major so the chunk-0 KV+Q0 projections stream
  against the DMA and the first scores fire as soon as chunk 0 lands.
- A dummy exp ACTIVATE at t=0 pulls the ~2.7us ACT table load into the
  DMA-startup shadow.
- Steady state: ACT does nothing but exp; all psum evacuations (y
  tiles, kT/vT/qT casts, epilogue) run on the DVE; tail o-proj casts
  alternate DVE/ACT.
- PSUM budget (8 banks): scores 2x[128,2,512] (4) + AV 2x[65,512] (2)
  + filler/o-proj mip ring x2 (2).  The mip ring is strict
  round-robin, so filler emission follows a parity-safe order: held
  projection accumulators alternate slots and transients come in
  windows where the previous occupant is free.

Layouts per core (S=2048, D=2048, 4 heads of 64):
  xT_sb  [128, 16, 2048] bf16   x^T k-tiles (host partition-major)
  qT_sb  [128, 2, 2048]  bf16   Q^T; head h -> partitions 64*(h%2), slot h//2
  kT_sb  [128, 2048]     bf16   K^T duplicated on both partition halves
  v1_sb  [128, 16, 65]   bf16   [V | ones] natural layout per sk tile
  oT_sb  [128, 2, 2048]  bf16   normalized attention out (same map as qT)
  out    [2048, 2048]    bf16   partial output, host-summed
"""

import numpy as np

S = 2048
D = 2048
N_CORES = 8
HD = 64
HPG = 4
QDIM = HPG * HD           # 256
SCALE = 1.0 / 8.0         # 1/sqrt(HD)
SQC = 512                 # seq chunk (psum bank width in f32)
NCH = S // SQC            # 4
T = S // 128              # 16 sk tiles
KO = D // 128             # 16 contraction tiles
QT = QDIM // 128          # 2 q partition tiles (= head pairs)

_compiled = {}


def _noldw(bi):
    bi.ins.ldweights = False
    return bi


def build_gqa(debug=False):
    import concourse.tile as tile
    from concourse import bacc, mybir
    from concourse.masks import make_identity
    from contextlib import ExitStack

    f32 = mybir.dt.float32
    bf16 = mybir.dt.bfloat16
    EXP = mybir.ActivationFunctionType.Exp

    nc = bacc.Bacc(None, target_bir_lowering=False, debug=debug)
    xTp = nc.declare_dram_parameter("xTp", [128, KO * S], bf16, isOutput=False)
    wqp0 = nc.declare_dram_parameter("wqp0", [128, KO * 128], bf16, isOutput=False)
    wqp1 = nc.declare_dram_parameter("wqp1", [128, KO * 128], bf16, isOutput=False)
    wkvp = nc.declare_dram_parameter("wkvp", [128, KO * 2 * HD], bf16, isOutput=False)
    wop = nc.declare_dram_parameter("wop", [128, QT * D], bf16, isOutput=False)
    out = nc.declare_dram_parameter("out", [S, D], bf16, isOutput=True)

    with tile.TileContext(nc) as tc, ExitStack() as ctx:
        const = ctx.enter_context(tc.tile_pool(name="const", bufs=1))
        persist = ctx.enter_context(tc.tile_pool(name="persist", bufs=1))

        ident = const.tile([128, 128], bf16)
        ones_bf = const.tile([1, HD], bf16)
        bias_exp = const.tile([128, 1], f32)
        warm = const.tile([128, 1], f32)
        # dummy exp ASAP: forces the ACT table load at t~0, under the DMA
        # startup shadow (otherwise it serializes before the first real exp)
        nc.vector.memset(bias_exp, -8.0)
        nc.scalar.activation(out=warm, in_=bias_exp, func=EXP, bias=0.0, scale=1.0)
        make_identity(nc, ident)
        nc.vector.memset(ones_bf, 1.0)

        xT_sb = persist.tile([128, KO, S], bf16)
        qT_sb = persist.tile([128, QT, S], bf16)
        kT_sb = persist.tile([128, S], bf16)
        v1_sb = persist.tile([128, T, HD + 1], bf16)
        oT_sb = persist.tile([128, QT, S], bf16)
        wkv_sb = persist.tile([128, KO, 2 * HD], bf16)
        wq_sb = persist.tile([128, QT, KO, 128], bf16)
        wo_sb = persist.tile([128, QT, D], bf16)
        vT_tmp = persist.tile([64, S], bf16)

        nc.vector.memset(v1_sb[:, :, HD:HD + 1], 1.0)

        # ---------------- input DMAs ------------------------------------
        # seq-chunk-major so compute can stream against DMA arrival.
        # Few, large descriptors (sync-engine issue is ~0.6us each).
        xTr = xTp[:].rearrange("p (ko s) -> p ko s", ko=KO)

        def dma_x(ch, ko_lo, ko_hi):
            cs = slice(ch * SQC, (ch + 1) * SQC)
            nc.sync.dma_start(
                out=xT_sb[:, ko_lo:ko_hi, cs], in_=xTr[:, ko_lo:ko_hi, cs])

        nc.sync.dma_start(out=wkv_sb, in_=wkvp[:].rearrange("p (ko m) -> p ko m", ko=KO))
        nc.sync.dma_start(
            out=wq_sb[:, 0], in_=wqp0[:].rearrange("p (ko m) -> p ko m", ko=KO))
        for g in range(4):                      # chunk 0 in 4-ko sub-DMAs
            dma_x(0, 4 * g, 4 * g + 4)
        dma_x(1, 0, 8)
        dma_x(1, 8, KO)
        nc.sync.dma_start(
            out=wq_sb[:, 1], in_=wqp1[:].rearrange("p (ko m) -> p ko m", ko=KO))
        dma_x(2, 0, 8)
        dma_x(2, 8, KO)
        dma_x(3, 0, 8)
        dma_x(3, 8, KO)
        nc.sync.dma_start(out=wo_sb, in_=wop[:].rearrange("p (qt m) -> p qt m", qt=QT))

        # ---------------- pools -----------------------------------------
        scps = ctx.enter_context(tc.tile_pool(name="scps", bufs=2, space="PSUM"))
        avps = ctx.enter_context(tc.tile_pool(name="avps", bufs=2, space="PSUM"))
        mips = ctx.enter_context(tc.tile_pool(name="mips", bufs=2, space="PSUM"))
        eps = ctx.enter_context(tc.tile_pool(name="eps", bufs=8))
        p2ev = ctx.enter_context(tc.tile_pool(name="p2ev", bufs=4))
        ypool = ctx.enter_context(tc.tile_pool(name="ypool", bufs=4))

        # ---------------- filler building blocks ------------------------
        proj_state = {}

        def kv_slice(ch, kos):
            cs = slice(ch * SQC, (ch + 1) * SQC)
            if kos[0] == 0:
                proj_state[("kv", ch)] = mips.tile(
                    [128, SQC], f32, name=f"kv{ch}", tag="mip")
            ps = proj_state[("kv", ch)]
            for ko in kos:
                nc.tensor.matmul(
                    ps, wkv_sb[:, ko, :], xT_sb[:, ko, cs],
                    start=(ko == 0), stop=(ko == KO - 1))

        def kv_cast(ch):
            cs = slice(ch * SQC, (ch + 1) * SQC)
            ps = proj_state.pop(("kv", ch))
            nc.vector.tensor_copy(out=kT_sb[0:64, cs], in_=ps[0:64, :])
            nc.vector.tensor_copy(out=kT_sb[64:128, cs], in_=ps[0:64, :])
            nc.vector.tensor_copy(out=vT_tmp[:, cs], in_=ps[64:128, :])

        def q_slice(qt, ch, kos):
            cs = slice(ch * SQC, (ch + 1) * SQC)
            if kos[0] == 0:
                proj_state[("q", qt, ch)] = mips.tile(
                    [128, SQC], f32, name=f"q{qt}{ch}", tag="mip")
            ps = proj_state[("q", qt, ch)]
            for ko in kos:
                nc.tensor.matmul(
                    ps, wq_sb[:, qt, ko, :], xT_sb[:, ko, cs],
                    start=(ko == 0), stop=(ko == KO - 1))

        def q_cast(qt, ch):
            cs = slice(ch * SQC, (ch + 1) * SQC)
            ps = proj_state.pop(("q", qt, ch))
            nc.vector.tensor_copy(out=qT_sb[:, qt, cs], in_=ps)

        def vtrans(j):
            pt = mips.tile([128, HD], bf16, name="pt", tag="mip")
            nc.tensor.transpose(
                pt, vT_tmp[:, j * 128:(j + 1) * 128], ident[0:64, 0:64])
            nc.vector.tensor_copy(out=v1_sb[:, j, 0:HD], in_=pt)

        def oproj(t, oc, tail_idx=-1):
            ns = slice(oc * SQC, (oc + 1) * SQC)
            if tail_idx >= 0:
                pool, tag = [(mips, "mip"), (avps, "av"), (scps, "sc")][tail_idx % 3]
                py = pool.tile([128, SQC], f32, name="py", tag=tag)
            else:
                py = mips.tile([128, SQC], f32, name="py", tag="mip")
            for qt in range(QT):
                nc.tensor.matmul(
                    py, oT_sb[:, qt, t * 128:(t + 1) * 128], wo_sb[:, qt, ns],
                    start=(qt == 0), stop=(qt == QT - 1))
            y_sb = ypool.tile([128, SQC], bf16, name="y_sb")
            with nc.allow_low_precision(reason="bf16 partial output"):
                if tail_idx >= 0 and tail_idx % 2 == 1:
                    nc.scalar.copy(out=y_sb, in_=py)
                else:
                    nc.vector.tensor_copy(out=y_sb, in_=py)
            nc.sync.dma_start(
                out=out[:].rearrange("(t p) n -> p t n", p=128)[:, t, ns],
                in_=y_sb)

        # ---------------- static filler schedule ------------------------
        # pre[(item, sk)] runs BEFORE the slot's score-lookahead emit (so
        # kv casts land ahead of the scores that read them — the PE FIFO
        # is in-order, a later-emitted dependency would deadlock it).
        # post[(item, sk)] runs after the slot's AV matmuls.
        pre, post = {}, {}

        def addp(d, item, sk, fn):
            d.setdefault((item, sk), []).append(fn)

        # kv chunks 1-3: front-loaded, DMA-gated (x chunk c lands ~17+5.4c us)
        addp(pre, 0, 0, lambda: kv_slice(1, range(0, 8)))
        addp(pre, 0, 1, lambda: kv_slice(1, range(8, KO)))
        addp(pre, 0, 1, lambda: kv_cast(1))
        addp(pre, 0, 4, lambda: kv_slice(2, range(0, 8)))
        addp(pre, 0, 5, lambda: kv_slice(2, range(8, KO)))
        addp(pre, 0, 6, lambda: kv_cast(2))
        addp(pre, 0, 8, lambda: kv_slice(3, range(0, 8)))
        addp(pre, 0, 9, lambda: kv_slice(3, range(8, KO)))
        addp(pre, 0, 10, lambda: kv_cast(3))
        # V transposes: v1[j] must exist before AV at (0, j)
        for j in range(4, 8):
            addp(post, 0, 1, lambda j=j: vtrans(j))
        for j in range(8, 12):
            addp(post, 0, 6, lambda j=j: vtrans(j))
        for j in range(12, 16):
            addp(post, 0, 10, lambda j=j: vtrans(j))
        # Q projections for later items (parity-safe slots, see docstring)
        addp(post, 0, 11, lambda: q_slice(1, 0, range(0, 6)))
        addp(post, 0, 12, lambda: q_slice(1, 0, range(6, 12)))
        addp(post, 0, 13, lambda: q_slice(1, 0, range(12, KO)))
        addp(post, 0, 13, lambda: q_cast(1, 0))
        addp(post, 0, 12, lambda: q_slice(0, 1, range(0, 6)))
        addp(post, 0, 13, lambda: q_slice(0, 1, range(6, 12)))
        addp(post, 0, 14, lambda: q_slice(0, 1, range(12, KO)))
        addp(post, 0, 14, lambda: q_cast(0, 1))
        addp(post, 1, 1, lambda: q_slice(1, 1, range(0, 6)))
        addp(post, 1, 2, lambda: q_slice(1, 1, range(6, 12)))
        addp(post, 1, 3, lambda: q_slice(1, 1, range(12, KO)))
        addp(post, 1, 3, lambda: q_cast(1, 1))
        addp(post, 1, 6, lambda: q_slice(0, 2, range(0, 6)))
        addp(post, 1, 7, lambda: q_slice(0, 2, range(6, 12)))
        addp(post, 1, 8, lambda: q_slice(0, 2, range(12, KO)))
        addp(post, 1, 8, lambda: q_cast(0, 2))
        addp(post, 2, 1, lambda: q_slice(1, 2, range(0, 6)))
        addp(post, 2, 2, lambda: q_slice(1, 2, range(6, 12)))
        addp(post, 2, 3, lambda: q_slice(1, 2, range(12, KO)))
        addp(post, 2, 3, lambda: q_cast(1, 2))
        addp(post, 3, 1, lambda: q_slice(0, 3, range(0, 6)))
        addp(post, 3, 2, lambda: q_slice(0, 3, range(6, 12)))
        addp(post, 3, 3, lambda: q_slice(0, 3, range(12, KO)))
        addp(post, 3, 3, lambda: q_cast(0, 3))
        addp(post, 4, 1, lambda: q_slice(1, 3, range(0, 6)))
        addp(post, 4, 2, lambda: q_slice(1, 3, range(6, 12)))
        addp(post, 4, 3, lambda: q_slice(1, 3, range(12, KO)))
        addp(post, 4, 3, lambda: q_cast(1, 3))
        # o-proj: chunk c's 16 tasks spread over item 5+c (one per sk)
        for ch in range(NCH - 1):
            tasks = [(4 * ch + tt, oc) for tt in range(4) for oc in range(NCH)]
            for i, (t, oc) in enumerate(tasks):
                addp(post, 5 + ch, i, lambda t=t, oc=oc: oproj(t, oc))

        # ---------------- prologue: chunk-0 KV + Q0, DMA-paced ----------
        for g in range(4):
            kv_slice(0, range(4 * g, 4 * g + 4))
            q_slice(0, 0, range(4 * g, 4 * g + 4))
        kv_cast(0)
        q_cast(0, 0)
        for j in range(4):
            vtrans(j)

        # ---------------- main loop: one flat pipeline -------------------
        items = [(ch, qt, sk)
                 for ch in range(NCH) for qt in range(QT) for sk in range(T)]
        sc_tiles = {}

        def emit_scores(idx):
            ch, qt, sk = items[idx]
            cs = slice(ch * SQC, (ch + 1) * SQC)
            sc = scps.tile([128, 2, SQC], f32, name="sc", tag="sc")
            for hh in range(2):
                hp = 64 * hh
                nc.tensor.matmul(
                    sc[:, hh, :],
                    kT_sb[hp:hp + 64, sk * 128:(sk + 1) * 128],
                    qT_sb[hp:hp + 64, qt, cs],
                    start=True, stop=True)
            sc_tiles[idx] = sc

        def evacuate_av(av):
            den, orw = [None, None], [None, None]
            for hh in range(2):
                den[hh] = p2ev.tile([1, SQC], f32, name=f"den{hh}", tag=f"den{hh}")
                nc.vector.tensor_copy(out=den[hh], in_=av[hh][HD:HD + 1, :])
                orw[hh] = p2ev.tile([HD, SQC], bf16, name=f"orw{hh}", tag=f"orw{hh}")
                with nc.allow_low_precision(reason="bf16 attn out"):
                    nc.vector.tensor_copy(out=orw[hh], in_=av[hh][0:HD, :])
            return den, orw

        def make_epilogue(ch, qt, den, orw):
            cs = slice(ch * SQC, (ch + 1) * SQC)

            def epi():
                for hh in range(2):
                    rf = p2ev.tile([1, SQC], f32, name=f"rf{hh}", tag=f"rf{hh}")
                    with nc.allow_low_precision(reason="softmax recip ~51ulp"):
                        nc.vector.reciprocal_approx_fast(out=rf, in_=den[hh])
                    rec = p2ev.tile([1, SQC], bf16, name=f"rec{hh}", tag=f"rec{hh}")
                    with nc.allow_low_precision(reason="bf16 recip bcast"):
                        nc.vector.tensor_copy(out=rec, in_=rf)
                    bc = mips.tile([128, SQC], f32, name="bc", tag="mip")
                    nc.tensor.matmul(
                        bc[0:HD, :], ones_bf, rec, start=True, stop=True)
                    bc_sb = p2ev.tile([HD, SQC], bf16, name=f"bcs{hh}", tag=f"bcs{hh}")
                    with nc.allow_low_precision(reason="bf16 recip bcast"):
                        nc.vector.tensor_copy(out=bc_sb, in_=bc[0:HD, :])
                    with nc.allow_low_precision(reason="bf16 attn out"):
                        nc.vector.tensor_mul(
                            out=oT_sb[64 * hh:64 * hh + 64, qt, cs],
                            in0=orw[hh], in1=bc_sb)
            return epi

        pending_epi = None
        av = None
        for idx, (ch, qt, sk) in enumerate(items):
            item = 2 * ch + qt
            for fn in pre.get((item, sk), ()):
                fn()
            if sk == 0:
                if idx == 0:
                    emit_scores(0)
                    emit_scores(1)
                av = [avps.tile([HD + 1, SQC], f32, name=f"av{hh}", tag="av")
                      for hh in range(2)]
            e_sb = eps.tile([128, 2, SQC], bf16, name="e_sb")
            nc.scalar.activation(
                out=e_sb, in_=sc_tiles.pop(idx),
                func=EXP, bias=bias_exp, scale=1.0)
            if idx + 2 < len(items):
                emit_scores(idx + 2)
            if pending_epi is not None:
                pending_epi()
                pending_epi = None
            for hh in range(2):
                bi = nc.tensor.matmul(
                    av[hh][:, :], v1_sb[:, sk, :], e_sb[:, hh, :],
                    start=(sk == 0), stop=(sk == T - 1))
                if hh:
                    _noldw(bi)
            for fn in post.get((item, sk), ()):
                fn()
            if sk == T - 1:
                den, orw = evacuate_av(av)
                pending_epi = make_epilogue(ch, qt, den, orw)
        pending_epi()
        # tail: o-proj of last chunk in waves of 6; casts alternate DVE/ACT
        base = (NCH - 1) * (SQC // 128)
        tasks = [(base + tt, oc) for tt in range(SQC // 128) for oc in range(NCH)]
        i = 0
        for w in range(0, len(tasks), 6):
            wave = tasks[w:w + 6]
            pys = []
            for t, oc in wave:
                ns = slice(oc * SQC, (oc + 1) * SQC)
                pool, tag = [(mips, "mip"), (avps, "av"), (scps, "sc")][i % 3]
                py = pool.tile([128, SQC], f32, name="py", tag=tag)
                for qt in range(QT):
                    nc.tensor.matmul(
                        py, oT_sb[:, qt, t * 128:(t + 1) * 128],
                        wo_sb[:, qt, ns],
                        start=(qt == 0), stop=(qt == QT - 1))
                pys.append(py)
                i += 1
            for (t, oc), py in zip(wave, pys):
                ns = slice(oc * SQC, (oc + 1) * SQC)
                y_sb = ypool.tile([128, SQC], bf16, name="y_sb")
                with nc.allow_low_precision(reason="bf16 partial output"):
                    if (t + oc) % 2 == 1:
                        nc.scalar.copy(out=y_sb, in_=py)
                    else:
                        nc.vector.tensor_copy(out=y_sb, in_=py)
                nc.sync.dma_start(
                    out=out[:].rearrange("(t p) n -> p t n", p=128)[:, t, ns],
                    in_=y_sb)

    nc.compile()
    return nc


def _get_nc():
    if "nc" not in _compiled:
        _compiled["nc"] = build_gqa()
    return _compiled["nc"]


def _pm(a):
    """[KO*128, M] -> partition-major [128, KO*M] (row p holds all ko chunks)."""
    ko = a.shape[0] // 128
    return np.ascontiguousarray(
        a.reshape(ko, 128, a.shape[1]).transpose(1, 0, 2).reshape(128, -1))


def _shard_inputs(x, w_q, w_k, w_v, w_o):
    import ml_dtypes

    bf = ml_dtypes.bfloat16
    x = np.asarray(x, dtype=np.float32)
    w_q = np.asarray(w_q, dtype=np.float32)
    w_k = np.asarray(w_k, dtype=np.float32)
    w_v = np.asarray(w_v, dtype=np.float32)
    w_o = np.asarray(w_o, dtype=np.float32)
    xT = np.ascontiguousarray(x.reshape(S, D).T)
    xTp = _pm(xT).astype(bf)
    in_maps = []
    for c in range(N_CORES):
        wkv = np.concatenate(
            [w_k[:, c * HD:(c + 1) * HD], w_v[:, c * HD:(c + 1) * HD]], axis=1)
        wq = w_q[:, c * QDIM:(c + 1) * QDIM] * np.float32(SCALE)
        in_maps.append({
            "xTp": xTp,
            "wqp0": _pm(wq[:, 0:128]).astype(bf),
            "wqp1": _pm(wq[:, 128:256]).astype(bf),
            "wkvp": _pm(wkv).astype(bf),
            "wop": _pm(w_o[c * QDIM:(c + 1) * QDIM, :]).astype(bf),
        })
    return in_maps


def kernel(x, w_q, w_k, w_v, w_o):
    from concourse.bass_utils import run_bass_kernel_spmd

    nc = _get_nc()
    in_maps = _shard_inputs(x, w_q, w_k, w_v, w_o)
    res = run_bass_kernel_spmd(nc, in_maps, list(range(N_CORES)))
    acc = np.zeros((S, D), dtype=np.float64)
    for r in res.results:
        acc += r["out"].astype(np.float64)
    return acc.astype(np.float32).reshape(1, S, D)
